# revision 4
# baseline (speedup 1.0000x reference)
"""Cross-attention kernel for Trainium2 (8 NeuronCores, SPMD data-parallel).

Problem: B=4, C=128, 64x64 spatial (N=4096 tokens), 4 heads of dim 32.
  q = Wq @ query; k = Wk @ key; v = Wv @ key   (1x1 convs == channel matmuls)
  out = softmax(q^T k / sqrt(32)) @ v          (per batch*head)

Sharding: 16 (batch, head) jobs -> 2 per core. Core i handles batch i//2,
heads {2*(i%2), 2*(i%2)+1} i.e. output channels [64*(i%2), 64*(i%2)+64).

On-chip layout ("scoresT"): scores are computed transposed, [nk, nq], so that
the PV matmul needs no transposes and the context comes out directly in
channel-major [d, nq] output layout. The softmax denominator is computed by
appending a ones-column to v^T (rides the PV accumulation for free); the
final division happens on the host (softmax is scale-invariant so exp-max
subtraction is unnecessary: scores ~ N(0,1)).

QK^T runs in fp8 (e4m3) with MatmulPerfMode.DoubleRow: 0.5 cycles/row, i.e.
2x bf16 streaming rate. Accuracy is recovered by error compensation: with
q = q8 + qr and k = k8 + kr (fp8 value + fp8 residual), the two DoubleRow
accumulation slots compute
    slot0: [k8; k8]^T [q8; qr] = k8^T q8 + k8^T qr
    slot1: [kr;  0]^T [q8; * ] = kr^T q8
dropping only the O(3e-4) kr^T qr term. Measured score error: ~2.4e-3
(natural log domain) vs ~0.037 for uncompensated fp8.

PV stays bf16 (fp8 probs would add ~3.4% output error; the DoubleRow slots
can't carry a probs residual without an extra elementwise pass, and the
PSUM->SBUF drain engines are the other near-critical resource).

exp() is split between ScalarE (exact spline exp -> bf16) and VectorE
(single-op Schraudolph exp2: int16 <- y*128 + B, bits reinterpreted as bf16),
because the PSUM->SBUF drain of the 33.5M score elements per core is a
throughput floor shared by only these two engines (GPSIMD can't read PSUM).
"""

import functools
import math

import numpy as np

NCORES = 8
B, C, HS, WS = 4, 128, 64, 64
N = HS * WS  # 4096 tokens
NUM_HEADS = 4
DH = 32  # head dim
HPC = 2  # heads per core

NQB = 512  # nq block (PSUM bank = 512 f32)
NKC = 128  # nk chunk (matmul M tile)
N_BLOCKS = N // NQB  # 8
N_CHUNKS = N // NKC  # 32
VTW = 66  # v^T tile width: 32 v cols + 1 ones col + pad to >64 so that
#            round_up(M)=128 keeps every matmul in the same 128x128 PE mode

# Schraudolph exp2 in bf16: i16 = cvt(y*128 + (16256 - C)); bits = bf16 ~ 2^y
# scores are pre-scaled by log2(e)/sqrt(DH) on the Wq side so y is in log2
# domain. ACT chunks use Exp with scale=ln(2) to undo the log2 scaling.
EXP2_A = 128.0
EXP2_B = 16256.0 - 5.25
N_ACT = 8  # of the 16 score tiles per (h, b): 8 exact-exp (ACT), 8 DVE


def _f32(x):
    return np.ascontiguousarray(np.asarray(x, dtype=np.float32))


def _bf16(x):
    import ml_dtypes

    return np.ascontiguousarray(np.asarray(x, dtype=np.float32).astype(ml_dtypes.bfloat16))


@functools.lru_cache(maxsize=1)
def _build_program():
    from contextlib import ExitStack

    import concourse.tile as tile
    from concourse import bacc, mybir
    from concourse.bass import ts

    f32 = mybir.dt.float32
    bf16 = mybir.dt.bfloat16
    fp8 = mybir.dt.float8e4
    i16 = mybir.dt.int16
    AF = mybir.ActivationFunctionType
    ALU = mybir.AluOpType
    DR = mybir.MatmulPerfMode.DoubleRow

    nc = bacc.Bacc(
        "TRN2",
        target_bir_lowering=False,
        debug=False,
        enable_asserts=False,
        num_devices=NCORES,
    )

    qin = nc.dram_tensor("qin", [128, N], bf16, kind="ExternalInput").ap()
    kin = nc.dram_tensor("kin", [128, N], bf16, kind="ExternalInput").ap()
    # w_all: projection stationary [c_in=128, c_out=128] (host-prepared):
    # cols 0-31 q_h0 (scaled by log2e/sqrt(DH)), 32-63 q_h1, 64-95 k_h0,
    # 96-127 k_h1.
    w_all = nc.dram_tensor("w_all", [128, 128], bf16, kind="ExternalInput").ap()
    wv_t = nc.dram_tensor("wv_t", [128, 64], bf16, kind="ExternalInput").ap()

    out_ctx = nc.dram_tensor("out_ctx", [64, N], f32, kind="ExternalOutput").ap()
    out_den = nc.dram_tensor("out_den", [2, N], f32, kind="ExternalOutput").ap()

    with tile.TileContext(nc) as tc, ExitStack() as ctx:
        persist = ctx.enter_context(tc.tile_pool(name="persist", bufs=1))

        # ---- load inputs ----
        wa_sb = persist.tile([128, 128], bf16)
        wv_sb = persist.tile([128, 64], bf16)
        nc.sync.dma_start(out=wa_sb, in_=w_all)
        nc.sync.dma_start(out=wv_sb, in_=wv_t)

        # ---- fp8 q/k structures ----
        # qmov (moving): rows 64h..64h+32 slot0 = q8_h, rows +32..64 slot0 =
        # qr_h, slot1 = [q8_h; 0].
        # kst (stationary): rows 64h..+32 slot0 = k8_h, +32..64 slot0 = k8_h
        # again; slot1 = [kr_h; 0].
        qmov = persist.tile([128, 2, N], fp8, name="qmov")
        kst = persist.tile([128, 2, N], fp8, name="kst")
        nc.gpsimd.memset(qmov[:, 1, :], 0.0)
        nc.gpsimd.memset(kst[:, 1, :], 0.0)

        # v^T per head: chunk c occupies cols [c*VTW, c*VTW+32) (nk on
        # partitions), col c*VTW+32 is the ones column for the denominator.
        vt = [
            persist.tile([128, VTW * N_CHUNKS], bf16, name=f"vt{h}")
            for h in range(HPC)
        ]
        for h in range(HPC):
            nc.gpsimd.memset(vt[h], 1.0)

        # One shared PSUM pool for projection outputs and attention score
        # tiles (same 2-bank slot size).
        sc_pool = ctx.enter_context(tc.tile_pool(name="sc", bufs=3, space="PSUM"))
        ctx_pool = ctx.enter_context(tc.tile_pool(name="ctxp", bufs=2, space="PSUM"))
        ex_pool = ctx.enter_context(tc.tile_pool(name="ex", bufs=8))
        out_pool = ctx.enter_context(tc.tile_pool(name="outp", bufs=4))

        with tc.tile_pool(name="inp", bufs=1) as inp_pool:
            qin_sb = inp_pool.tile([128, N], bf16)
            kin_sb = inp_pool.tile([128, N], bf16)
            # alternate chunks across the HWDGE (sync) and SWDGE (gpsimd)
            # queues so q and k stream in concurrently
            for t in range(4):
                eng_q = nc.sync if t % 2 == 0 else nc.gpsimd
                eng_k = nc.gpsimd if t % 2 == 0 else nc.sync
                eng_q.dma_start(
                    out=qin_sb[:, ts(t, N // 4)], in_=qin[:, ts(t, N // 4)]
                )
                eng_k.dma_start(
                    out=kin_sb[:, ts(t, N // 4)], in_=kin[:, ts(t, N // 4)]
                )

            pp = sc_pool
            # ---- q/k projections + fp8 drain ----
            # psum rows: 0-31 q'_h0, 32-63 q'_h1, 64-95 k_h0, 96-127 k_h1
            for t2 in range(N // (2 * NQB)):
                qk = pp.tile([128, 2 * NQB], f32, name="qk", tag="sc")
                for j in range(2):
                    cols = ts(2 * t2 + j, NQB)
                    nc.tensor.matmul(
                        out=qk[0:64, ts(j, NQB)],
                        lhsT=wa_sb[:, 0:64],
                        rhs=qin_sb[:, cols],
                        start=True,
                        stop=True,
                    )
                    nc.tensor.matmul(
                        out=qk[64:128, ts(j, NQB)],
                        lhsT=wa_sb[:, 64:128],
                        rhs=kin_sb[:, cols],
                        start=True,
                        stop=True,
                    )
                dcols = ts(t2, 2 * NQB)
                for h in range(HPC):
                    qrow, krow = 32 * h, 64 + 32 * h
                    p0 = 64 * h
                    # fp8 value (ACT) then residual (DVE; PSUM+SBUF inputs)
                    nc.scalar.copy(qmov[p0 : p0 + 32, 0, dcols], qk[qrow : qrow + 32, :])
                    nc.scalar.copy(kst[p0 : p0 + 32, 0, dcols], qk[krow : krow + 32, :])
                    nc.vector.tensor_tensor(
                        qmov[p0 + 32 : p0 + 64, 0, dcols],
                        qk[qrow : qrow + 32, :],
                        qmov[p0 : p0 + 32, 0, dcols],
                        op=ALU.subtract,
                    )
                    nc.vector.tensor_tensor(
                        kst[p0 : p0 + 32, 1, dcols],
                        qk[krow : krow + 32, :],
                        kst[p0 : p0 + 32, 0, dcols],
                        op=ALU.subtract,
                    )
                    # duplicates (SBUF->SBUF, Pool engine)
                    nc.gpsimd.tensor_copy(
                        qmov[p0 : p0 + 32, 1, dcols], qmov[p0 : p0 + 32, 0, dcols]
                    )
                    nc.gpsimd.tensor_copy(
                        kst[p0 + 32 : p0 + 64, 0, dcols], kst[p0 : p0 + 32, 0, dcols]
                    )

            # ---- v^T: out[nk, c_out] = sum_c key[c, nk] * Wv[c_out, c] ----
            # 16 chunks of [128, 64] fill one 2-bank psum tile; copy per head.
            for g in range(N_CHUNKS // 16):
                vp = pp.tile([128, 2 * NQB], f32, name="vp", tag="sc")
                for j in range(16):
                    c = g * 16 + j
                    nc.tensor.matmul(
                        out=vp[:, ts(j, 64)],
                        lhsT=kin_sb[:, ts(c, NKC)],
                        rhs=wv_sb,
                        start=True,
                        stop=True,
                    )
                vp3 = vp.rearrange("p (j w) -> p j w", j=16)
                for h in range(HPC):
                    dst = vt[h][:, g * 16 * VTW : (g + 1) * 16 * VTW]
                    dst3 = dst.rearrange("p (j w) -> p j w", j=16)
                    eng = nc.vector.tensor_copy if h % 2 == 0 else nc.scalar.copy
                    eng(dst3[:, :, 0:32], vp3[:, :, ts(h, 32)])

        # ---- attention ----
        # sc tiles span 2 PSUM banks (2 nk chunks); one exp instruction
        # drains both chunks to amortize the per-op overhead.
        #
        # Software-pipelined: the QK matmuls for score tile i are emitted
        # BEFORE the exp+PV of tile i-1, so the (in-order) PE always has an
        # independent QK pair to chew on while ACT/DVE drain the previous
        # tile. Without this the PE serializes on exp and idles ~40%.
        ln2 = math.log(2.0)
        pending = None  # (sc, h, b, c2, ctx_ps) awaiting exp+PV

        def flush_pending():
            sc, h, b, c2, ctx_ps = pending
            ex = ex_pool.tile([128, 2 * NQB], bf16, name="ex")
            if c2 >= N_ACT:
                # fast exp2 on DVE: i16 <- sc*128 + bias; bits = bf16
                nc.vector.tensor_scalar(
                    ex.bitcast(i16), sc, EXP2_A, EXP2_B,
                    op0=ALU.mult, op1=ALU.add,
                )
            else:
                # exact exp on ACT: exp(ln2 * y) = 2^y
                nc.scalar.activation(ex, sc, AF.Exp, scale=ln2)
            for j in range(2):
                # ctxT[d, nq] += v^T[d, nk] @ probsT[nk, nq]; row 32
                # accumulates the softmax denominator (ones column).
                c = 2 * c2 + j
                nc.tensor.matmul(
                    out=ctx_ps,
                    lhsT=vt[h][:, c * VTW : (c + 1) * VTW],
                    rhs=ex[:, ts(j, NQB)],
                    start=(c == 0),
                    stop=(c == N_CHUNKS - 1),
                )
            if c2 == N_CHUNKS // 2 - 1:
                ob = out_pool.tile([33, NQB], f32, name="ob")
                nc.scalar.copy(ob, ctx_ps[0:33, :])
                nc.sync.dma_start(out=out_ctx[ts(h, 32), ts(b, NQB)], in_=ob[0:32, :])
                nc.sync.dma_start(
                    out=out_den[h : h + 1, ts(b, NQB)], in_=ob[32:33, :]
                )

        for h in range(HPC):
            p0 = 64 * h
            for b in range(N_BLOCKS):
                ctx_ps = ctx_pool.tile([VTW, NQB], f32, name="ctx_ps")
                rhs_q = qmov[p0 : p0 + 64, :, ts(b, NQB)]
                for c2 in range(N_CHUNKS // 2):
                    sc = sc_pool.tile([128, 2 * NQB], f32, name="sc")
                    for j in range(2):
                        c = 2 * c2 + j
                        # scoresT[nk_chunk, nq_block] via compensated fp8
                        nc.tensor.matmul(
                            out=sc[:, ts(j, NQB)],
                            lhsT=kst[p0 : p0 + 64, :, ts(c, NKC)],
                            rhs=rhs_q,
                            start=True,
                            stop=True,
                            perf_mode=DR,
                        )
                    if pending is not None:
                        flush_pending()
                    pending = (sc, h, b, c2, ctx_ps)
        flush_pending()

    nc.compile()
    return nc


def _shard_inputs(query, key, Wq, Wk, Wv):
    query = _f32(query).reshape(B, C, N)
    key = _f32(key).reshape(B, C, N)
    Wq, Wk, Wv = _f32(Wq), _f32(Wk), _f32(Wv)

    scale = math.log2(math.e) / math.sqrt(DH)
    in_maps = []
    for core in range(NCORES):
        b, half = core // 2, core % 2
        w_all = np.zeros((128, 128), np.float32)
        wv_t = np.zeros((128, 64), np.float32)
        for hl in range(HPC):
            ch0 = 64 * half + 32 * hl
            w_all[:, 32 * hl : 32 * hl + 32] = Wq[ch0 : ch0 + 32, :].T * scale
            w_all[:, 64 + 32 * hl : 64 + 32 * hl + 32] = Wk[ch0 : ch0 + 32, :].T
            wv_t[:, 32 * hl : 32 * hl + 32] = Wv[ch0 : ch0 + 32, :].T
        in_maps.append(
            {
                "qin": _bf16(query[b]),
                "kin": _bf16(key[b]),
                "w_all": _bf16(w_all),
                "wv_t": _bf16(wv_t),
            }
        )
    return in_maps


def _run(in_maps, trace=False):
    from concourse import bass_utils

    nc = _build_program()
    return bass_utils.run_bass_kernel_spmd(
        nc, in_maps, core_ids=list(range(NCORES)), trace=trace
    )


def _assemble(results):
    out = np.empty((B, C, N), np.float32)
    for core in range(NCORES):
        b, half = core // 2, core % 2
        r = results[core]
        ctx = r["out_ctx"]  # [64, N]
        den = r["out_den"]  # [2, N]
        for hl in range(HPC):
            out[b, 64 * half + 32 * hl : 64 * half + 32 * hl + 32, :] = (
                ctx[32 * hl : 32 * hl + 32, :] / den[hl][None, :]
            )
    return out.reshape(B, C, HS, WS)


def kernel(query, key, Wq, Wk, Wv):
    in_maps = _shard_inputs(query, key, Wq, Wk, Wv)
    res = _run(in_maps)
    return _assemble(res.results)


# revision 8
# speedup vs baseline: 1.2885x; 1.2885x over previous
"""Cross-attention kernel for Trainium2 (8 NeuronCores, SPMD data-parallel).

Problem: B=4, C=128, 64x64 spatial (N=4096 tokens), 4 heads of dim 32.
  q = Wq @ query; k = Wk @ key; v = Wv @ key   (1x1 convs == channel matmuls)
  out = softmax(q^T k / sqrt(32)) @ v          (per batch*head)

Sharding: 16 (batch, head) jobs -> 2 per core. Core i handles batch i//2,
heads {2*(i%2), 2*(i%2)+1} i.e. output channels [64*(i%2), 64*(i%2)+64).

On-chip layout ("scoresT"): scores are computed transposed, [nk, nq], so that
the PV matmul needs no transposes and the context comes out directly in
channel-major [d, nq] output layout. The softmax denominator is computed by
appending a ones-column to v^T (rides the PV accumulation for free); the
final division happens on the host (softmax is scale-invariant so exp-max
subtraction is unnecessary: scores ~ N(0,1)).

QK^T runs in fp8 (e4m3) with MatmulPerfMode.DoubleRow: 0.5 cycles/row, i.e.
2x bf16 streaming rate. Accuracy is recovered by error compensation: with
q = q8 + qr and k = k8 + kr (fp8 value + fp8 residual), the two DoubleRow
accumulation slots compute
    slot0: [k8; k8]^T [q8; qr] = k8^T q8 + k8^T qr
    slot1: [kr;  0]^T [q8; * ] = kr^T q8
dropping only the O(3e-4) kr^T qr term. Measured score error: ~2.4e-3
(natural log domain) vs ~0.037 for uncompensated fp8.

PV stays bf16 (fp8 probs would add ~3.4% output error; the DoubleRow slots
can't carry a probs residual without an extra elementwise pass, and the
PSUM->SBUF drain engines are the other near-critical resource).

exp() is split between ScalarE (exact spline exp -> bf16) and VectorE
(single-op Schraudolph exp2: int16 <- y*128 + B, bits reinterpreted as bf16),
because the PSUM->SBUF drain of the 33.5M score elements per core is a
throughput floor shared by only these two engines (GPSIMD can't read PSUM).
"""

import functools
import math

import numpy as np

NCORES = 8
B, C, HS, WS = 4, 128, 64, 64
N = HS * WS  # 4096 tokens
NUM_HEADS = 4
DH = 32  # head dim
HPC = 2  # heads per core

NQB = 512  # nq block (PSUM bank = 512 f32)
NKC = 128  # nk chunk (matmul M tile)
N_BLOCKS = N // NQB  # 8
N_CHUNKS = N // NKC  # 32
VTW = 66  # v^T tile width: 32 v cols + 1 ones col + pad to >64 so that
#            round_up(M)=128 keeps every matmul in the same 128x128 PE mode

# Schraudolph exp2 in bf16: i16 = cvt(y*128 + (16256 - C)); bits = bf16 ~ 2^y
# scores are pre-scaled by log2(e)/sqrt(DH) on the Wq side so y is in log2
# domain. ACT chunks use Exp with scale=ln(2) to undo the log2 scaling.
EXP2_A = 128.0
EXP2_B = 16256.0 - 5.25
N_ACT = 8  # of the 16 score tiles per (h, b): 8 exact-exp (ACT), 8 DVE


def _f32(x):
    return np.ascontiguousarray(np.asarray(x, dtype=np.float32))


def _bf16(x):
    import ml_dtypes

    return np.ascontiguousarray(np.asarray(x, dtype=np.float32).astype(ml_dtypes.bfloat16))


@functools.lru_cache(maxsize=1)
def _build_program():
    from contextlib import ExitStack

    import concourse.tile as tile
    from concourse import bacc, mybir
    from concourse.bass import ts

    f32 = mybir.dt.float32
    bf16 = mybir.dt.bfloat16
    fp8 = mybir.dt.float8e4
    i16 = mybir.dt.int16
    AF = mybir.ActivationFunctionType
    ALU = mybir.AluOpType
    DR = mybir.MatmulPerfMode.DoubleRow

    nc = bacc.Bacc(
        "TRN2",
        target_bir_lowering=False,
        debug=False,
        enable_asserts=False,
        num_devices=NCORES,
    )

    qin = nc.dram_tensor("qin", [128, N], bf16, kind="ExternalInput").ap()
    kin = nc.dram_tensor("kin", [128, N], bf16, kind="ExternalInput").ap()
    # w_all: projection stationary [c_in=128, c_out=128] (host-prepared):
    # cols 0-31 q_h0 (scaled by log2e/sqrt(DH)), 32-63 q_h1, 64-95 k_h0,
    # 96-127 k_h1.
    w_all = nc.dram_tensor("w_all", [128, 128], bf16, kind="ExternalInput").ap()
    wv_t = nc.dram_tensor("wv_t", [128, 64], bf16, kind="ExternalInput").ap()

    out_ctx = nc.dram_tensor("out_ctx", [64, N], f32, kind="ExternalOutput").ap()
    out_den = nc.dram_tensor("out_den", [2, N], f32, kind="ExternalOutput").ap()

    with tile.TileContext(nc) as tc, ExitStack() as ctx:
        persist = ctx.enter_context(tc.tile_pool(name="persist", bufs=1))

        # ---- load inputs ----
        wa_sb = persist.tile([128, 128], bf16)
        wv_sb = persist.tile([128, 64], bf16)
        nc.sync.dma_start(out=wa_sb, in_=w_all)
        nc.sync.dma_start(out=wv_sb, in_=wv_t)

        # ---- fp8 q/k structures ----
        # qmov (moving): rows 64h..64h+32 slot0 = q8_h, rows +32..64 slot0 =
        # qr_h, slot1 = [q8_h; 0].
        # kst (stationary): rows 64h..+32 slot0 = k8_h, +32..64 slot0 = k8_h
        # again; slot1 = [kr_h; 0].
        qmov = persist.tile([128, 2, N], fp8, name="qmov")
        kst = persist.tile([128, 2, N], fp8, name="kst")
        nc.gpsimd.memset(qmov[:, 1, :], 0.0)
        nc.gpsimd.memset(kst[:, 1, :], 0.0)
        # fp8 staging: qk8 = fp8(proj), qkr = fp8(proj - qk8), both in proj
        # row layout [q_h0, k_h0, q_h1, k_h1] so that ONE full-width convert
        # and ONE full-width residual per proj tile cover all four bands.
        qk8 = persist.tile([128, N], fp8, name="qk8")
        qkr = persist.tile([128, N], fp8, name="qkr")

        # v^T per head: chunk c occupies cols [c*VTW, c*VTW+32) (nk on
        # partitions), col c*VTW+32 is the ones column for the denominator.
        vt = [
            persist.tile([128, VTW * N_CHUNKS], bf16, name=f"vt{h}")
            for h in range(HPC)
        ]
        for h in range(HPC):
            nc.gpsimd.memset(vt[h], 1.0)

        # One shared PSUM pool for projection outputs and attention score
        # tiles (same 2-bank slot size).
        sc_pool = ctx.enter_context(tc.tile_pool(name="sc", bufs=3, space="PSUM"))
        ctx_pool = ctx.enter_context(tc.tile_pool(name="ctxp", bufs=2, space="PSUM"))
        ex_pool = ctx.enter_context(tc.tile_pool(name="ex", bufs=8))
        out_pool = ctx.enter_context(tc.tile_pool(name="outp", bufs=4))

        with tc.tile_pool(name="inp", bufs=1) as inp_pool:
            qin_sb = inp_pool.tile([128, N], bf16)
            kin_sb = inp_pool.tile([128, N], bf16)
            # alternate chunks across the HWDGE (sync) and SWDGE (gpsimd)
            # queues so q and k stream in concurrently
            for t in range(4):
                eng_q = nc.sync if t % 2 == 0 else nc.gpsimd
                eng_k = nc.gpsimd if t % 2 == 0 else nc.sync
                eng_q.dma_start(
                    out=qin_sb[:, ts(t, N // 4)], in_=qin[:, ts(t, N // 4)]
                )
                eng_k.dma_start(
                    out=kin_sb[:, ts(t, N // 4)], in_=kin[:, ts(t, N // 4)]
                )

            pp = sc_pool
            # ---- q/k projections + fp8 drain ----
            # psum rows: 0-31 q'_h0, 32-63 q'_h1, 64-95 k_h0, 96-127 k_h1
            for t2 in range(N // (2 * NQB)):
                qk = pp.tile([128, 2 * NQB], f32, name="qk", tag="sc")
                for j in range(2):
                    cols = ts(2 * t2 + j, NQB)
                    nc.tensor.matmul(
                        out=qk[0:64, ts(j, NQB)],
                        lhsT=wa_sb[:, 0:64],
                        rhs=qin_sb[:, cols],
                        start=True,
                        stop=True,
                    )
                    nc.tensor.matmul(
                        out=qk[64:128, ts(j, NQB)],
                        lhsT=wa_sb[:, 64:128],
                        rhs=kin_sb[:, cols],
                        start=True,
                        stop=True,
                    )
                dcols = ts(t2, 2 * NQB)
                # one convert + one residual for all four 32-row bands
                nc.scalar.copy(qk8[:, dcols], qk)
                nc.vector.tensor_tensor(
                    qkr[:, dcols], qk, qk8[:, dcols], op=ALU.subtract
                )
                # fan-out copies into the matmul operand structures
                # (SBUF->SBUF fp8; mostly on the otherwise-idle Pool engine)
                for h in range(HPC):
                    p0 = 64 * h
                    q8s = qk8[32 * h : 32 * h + 32, dcols]
                    qrs = qkr[32 * h : 32 * h + 32, dcols]
                    k8s = qk8[64 + 32 * h : 64 + 32 * h + 32, dcols]
                    krs = qkr[64 + 32 * h : 64 + 32 * h + 32, dcols]
                    eng = nc.gpsimd.tensor_copy
                    eng(qmov[p0 : p0 + 32, 0, dcols], q8s)
                    eng(qmov[p0 + 32 : p0 + 64, 0, dcols], qrs)
                    nc.scalar.copy(qmov[p0 : p0 + 32, 1, dcols], q8s)
                    eng(kst[p0 : p0 + 32, 0, dcols], k8s)
                    nc.vector.tensor_copy(kst[p0 + 32 : p0 + 64, 0, dcols], k8s)
                    eng(kst[p0 : p0 + 32, 1, dcols], krs)

            # ---- v^T: out[nk, c_out] = sum_c key[c, nk] * Wv[c_out, c] ----
            # 16 chunks of [128, 64] fill one 2-bank psum tile; copy per head.
            for g in range(N_CHUNKS // 16):
                vp = pp.tile([128, 2 * NQB], f32, name="vp", tag="sc")
                for j in range(16):
                    c = g * 16 + j
                    nc.tensor.matmul(
                        out=vp[:, ts(j, 64)],
                        lhsT=kin_sb[:, ts(c, NKC)],
                        rhs=wv_sb,
                        start=True,
                        stop=True,
                    )
                vp3 = vp.rearrange("p (j w) -> p j w", j=16)
                for h in range(HPC):
                    dst = vt[h][:, g * 16 * VTW : (g + 1) * 16 * VTW]
                    dst3 = dst.rearrange("p (j w) -> p j w", j=16)
                    eng = nc.vector.tensor_copy if h % 2 == 0 else nc.scalar.copy
                    eng(dst3[:, :, 0:32], vp3[:, :, ts(h, 32)])

        # ---- attention ----
        # sc tiles span 2 PSUM banks (2 nk chunks); one exp instruction
        # drains both chunks to amortize the per-op overhead.
        #
        # Software-pipelined depth 2: the QK matmuls for score tile i are
        # emitted BEFORE the exp+PV of tile i-2, so the (in-order) PE always
        # has independent QK work while ACT/DVE drain earlier tiles — the exp
        # drain latency (~1.3us incl. semaphore hops) spans two iterations of
        # PE work. exp alternates ACT (exact, even tiles) / DVE (Schraudolph,
        # odd tiles) so each engine gets two iterations per op.
        ln2 = math.log(2.0)
        from collections import deque

        pipeline = deque()  # (sc, h, b, c2, ctx_ps) awaiting exp+PV

        def flush_one():
            sc, h, b, c2, ctx_ps = pipeline.popleft()
            ex = ex_pool.tile([128, 2 * NQB], bf16, name="ex")
            if c2 % 2 == 1:
                # fast exp2 on DVE: i16 <- sc*128 + bias; bits = bf16
                nc.vector.tensor_scalar(
                    ex.bitcast(i16), sc, EXP2_A, EXP2_B,
                    op0=ALU.mult, op1=ALU.add,
                )
            else:
                # exact exp on ACT: exp(ln2 * y) = 2^y
                nc.scalar.activation(ex, sc, AF.Exp, scale=ln2)
            for j in range(2):
                # ctxT[d, nq] += v^T[d, nk] @ probsT[nk, nq]; row 32
                # accumulates the softmax denominator (ones column).
                c = 2 * c2 + j
                nc.tensor.matmul(
                    out=ctx_ps,
                    lhsT=vt[h][:, c * VTW : (c + 1) * VTW],
                    rhs=ex[:, ts(j, NQB)],
                    start=(c == 0),
                    stop=(c == N_CHUNKS - 1),
                )
            if c2 == N_CHUNKS // 2 - 1:
                ob = out_pool.tile([33, NQB], f32, name="ob")
                nc.scalar.copy(ob, ctx_ps[0:33, :])
                nc.sync.dma_start(out=out_ctx[ts(h, 32), ts(b, NQB)], in_=ob[0:32, :])
                nc.sync.dma_start(
                    out=out_den[h : h + 1, ts(b, NQB)], in_=ob[32:33, :]
                )

        for h in range(HPC):
            p0 = 64 * h
            for b in range(N_BLOCKS):
                ctx_ps = ctx_pool.tile([VTW, NQB], f32, name="ctx_ps")
                rhs_q = qmov[p0 : p0 + 64, :, ts(b, NQB)]
                for c2 in range(N_CHUNKS // 2):
                    sc = sc_pool.tile([128, 2 * NQB], f32, name="sc")
                    for j in range(2):
                        c = 2 * c2 + j
                        # scoresT[nk_chunk, nq_block] via compensated fp8
                        nc.tensor.matmul(
                            out=sc[:, ts(j, NQB)],
                            lhsT=kst[p0 : p0 + 64, :, ts(c, NKC)],
                            rhs=rhs_q,
                            start=True,
                            stop=True,
                            perf_mode=DR,
                        )
                    pipeline.append((sc, h, b, c2, ctx_ps))
                    if len(pipeline) > 2:
                        flush_one()
        while pipeline:
            flush_one()

    nc.compile()
    return nc


def _shard_inputs(query, key, Wq, Wk, Wv):
    query = _f32(query).reshape(B, C, N)
    key = _f32(key).reshape(B, C, N)
    Wq, Wk, Wv = _f32(Wq), _f32(Wk), _f32(Wv)

    scale = math.log2(math.e) / math.sqrt(DH)
    in_maps = []
    for core in range(NCORES):
        b, half = core // 2, core % 2
        w_all = np.zeros((128, 128), np.float32)
        wv_t = np.zeros((128, 64), np.float32)
        for hl in range(HPC):
            ch0 = 64 * half + 32 * hl
            w_all[:, 32 * hl : 32 * hl + 32] = Wq[ch0 : ch0 + 32, :].T * scale
            w_all[:, 64 + 32 * hl : 64 + 32 * hl + 32] = Wk[ch0 : ch0 + 32, :].T
            wv_t[:, 32 * hl : 32 * hl + 32] = Wv[ch0 : ch0 + 32, :].T
        in_maps.append(
            {
                "qin": _bf16(query[b]),
                "kin": _bf16(key[b]),
                "w_all": _bf16(w_all),
                "wv_t": _bf16(wv_t),
            }
        )
    return in_maps


def _run(in_maps, trace=False):
    from concourse import bass_utils

    nc = _build_program()
    return bass_utils.run_bass_kernel_spmd(
        nc, in_maps, core_ids=list(range(NCORES)), trace=trace
    )


def _assemble(results):
    out = np.empty((B, C, N), np.float32)
    for core in range(NCORES):
        b, half = core // 2, core % 2
        r = results[core]
        ctx = r["out_ctx"]  # [64, N]
        den = r["out_den"]  # [2, N]
        for hl in range(HPC):
            out[b, 64 * half + 32 * hl : 64 * half + 32 * hl + 32, :] = (
                ctx[32 * hl : 32 * hl + 32, :] / den[hl][None, :]
            )
    return out.reshape(B, C, HS, WS)


def kernel(query, key, Wq, Wk, Wv):
    in_maps = _shard_inputs(query, key, Wq, Wk, Wv)
    res = _run(in_maps)
    return _assemble(res.results)


# revision 13
# speedup vs baseline: 1.3806x; 1.0715x over previous
"""Cross-attention kernel for Trainium2 (8 NeuronCores, SPMD data-parallel).

Problem: B=4, C=128, 64x64 spatial (N=4096 tokens), 4 heads of dim 32.
  q = Wq @ query; k = Wk @ key; v = Wv @ key   (1x1 convs == channel matmuls)
  out = softmax(q^T k / sqrt(32)) @ v          (per batch*head)

Sharding: 16 (batch, head) jobs -> 2 per core. Core i handles batch i//2,
heads {2*(i%2), 2*(i%2)+1} i.e. output channels [64*(i%2), 64*(i%2)+64).

On-chip layout ("scoresT"): scores are computed transposed, [nk, nq], so that
the PV matmul needs no transposes and the context comes out directly in
channel-major [d, nq] output layout. The softmax denominator is computed by
appending a ones-column to v^T (rides the PV accumulation for free); the
final division happens on the host (softmax is scale-invariant so exp-max
subtraction is unnecessary: scores ~ N(0,1)).

QK^T runs in fp8 (e4m3) with MatmulPerfMode.DoubleRow: 0.5 cycles/row, i.e.
2x bf16 streaming rate. Accuracy is recovered by error compensation: with
q = q8 + qr and k = k8 + kr (fp8 value + fp8 residual), the two DoubleRow
accumulation slots compute
    slot0: [k8; k8]^T [q8; qr] = k8^T q8 + k8^T qr
    slot1: [kr;  0]^T [q8; * ] = kr^T q8
dropping only the O(3e-4) kr^T qr term. Measured score error: ~2.4e-3
(natural log domain) vs ~0.037 for uncompensated fp8.

PV stays bf16 (fp8 probs would add ~3.4% output error; the DoubleRow slots
can't carry a probs residual without an extra elementwise pass, and the
PSUM->SBUF drain engines are the other near-critical resource).

exp() is split between ScalarE (exact spline exp -> bf16) and VectorE
(single-op Schraudolph exp2: int16 <- y*128 + B, bits reinterpreted as bf16),
because the PSUM->SBUF drain of the 33.5M score elements per core is a
throughput floor shared by only these two engines (GPSIMD can't read PSUM).
"""

import functools
import math

import numpy as np

NCORES = 8
B, C, HS, WS = 4, 128, 64, 64
N = HS * WS  # 4096 tokens
NUM_HEADS = 4
DH = 32  # head dim
HPC = 2  # heads per core

NQB = 512  # nq block (PSUM bank = 512 f32)
NKC = 128  # nk chunk (matmul M tile)
N_BLOCKS = N // NQB  # 8
N_CHUNKS = N // NKC  # 32
VTW = 66  # v^T tile width: 32 v cols + 1 ones col + pad to >64 so that
#            round_up(M)=128 keeps every matmul in the same 128x128 PE mode

# Schraudolph exp2 in bf16: i16 = cvt(y*128 + (16256 - C)); bits = bf16 ~ 2^y
# scores are pre-scaled by log2(e)/sqrt(DH) on the Wq side so y is in log2
# domain. ACT chunks use Exp with scale=ln(2) to undo the log2 scaling.
EXP2_A = 128.0
EXP2_B = 16256.0 - 5.25
N_ACT = 8  # of the 16 score tiles per (h, b): 8 exact-exp (ACT), 8 DVE


def _f32(x):
    return np.ascontiguousarray(np.asarray(x, dtype=np.float32))


def _bf16(x):
    import ml_dtypes

    return np.ascontiguousarray(np.asarray(x, dtype=np.float32).astype(ml_dtypes.bfloat16))


@functools.lru_cache(maxsize=1)
def _build_program():
    from contextlib import ExitStack

    import concourse.tile as tile
    from concourse import bacc, mybir
    from concourse.bass import ts

    f32 = mybir.dt.float32
    bf16 = mybir.dt.bfloat16
    fp8 = mybir.dt.float8e4
    i16 = mybir.dt.int16
    AF = mybir.ActivationFunctionType
    ALU = mybir.AluOpType
    DR = mybir.MatmulPerfMode.DoubleRow

    nc = bacc.Bacc(
        "TRN2",
        target_bir_lowering=False,
        debug=False,
        enable_asserts=False,
        num_devices=NCORES,
    )

    qin = nc.dram_tensor("qin", [128, N], bf16, kind="ExternalInput").ap()
    kin = nc.dram_tensor("kin", [128, N], bf16, kind="ExternalInput").ap()
    # w_all: projection stationary [c_in=128, c_out=128] (host-prepared):
    # cols 0-31 q_h0 (scaled by log2e/sqrt(DH)), 32-63 q_h1, 64-95 k_h0,
    # 96-127 k_h1.
    w_all = nc.dram_tensor("w_all", [128, 128], bf16, kind="ExternalInput").ap()
    wv_t = nc.dram_tensor("wv_t", [128, 64], bf16, kind="ExternalInput").ap()

    # constant fills, DMA'd instead of burning engine memset time at startup
    zz = nc.dram_tensor("zz", [32, N], fp8, kind="ExternalInput").ap()
    vones = nc.dram_tensor(
        "vones", [128, VTW * N_CHUNKS], bf16, kind="ExternalInput"
    ).ap()

    out_ctx = nc.dram_tensor("out_ctx", [64, N], f32, kind="ExternalOutput").ap()
    out_den = nc.dram_tensor("out_den", [2, N], f32, kind="ExternalOutput").ap()

    with tile.TileContext(nc) as tc, ExitStack() as ctx:
        persist = ctx.enter_context(tc.tile_pool(name="persist", bufs=1))

        # ---- load inputs ----
        wa_sb = persist.tile([128, 128], bf16)
        wv_sb = persist.tile([128, 64], bf16)
        nc.sync.dma_start(out=wa_sb, in_=w_all)
        nc.sync.dma_start(out=wv_sb, in_=wv_t)

        # ---- fp8 q/k structures ----
        # qmov (moving): rows 64h..64h+32 = q8_h, rows +32..64 = qr_h; the
        # DoubleRow rhs broadcasts the slot dim with stride 0 (slot1's qr
        # rows meet zero rows of the stationary, so they're don't-cares).
        # kst (stationary): rows 64h..+32 slot0 = k8_h, +32..64 slot0 = k8_h
        # again; slot1 = [kr_h; 0] (zero bands DMA'd from zz).
        qmov = persist.tile([128, N], fp8, name="qmov")
        kst = persist.tile([128, 2, N], fp8, name="kst")
        nc.sync.dma_start(out=kst[32:64, 1, :], in_=zz)
        nc.sync.dma_start(out=kst[96:128, 1, :], in_=zz)
        # fp8 staging: qk8 = fp8(proj), qkr = fp8(proj - qk8), both in proj
        # row layout [q_h0, q_h1, k_h0, k_h1] so that ONE full-width convert
        # and ONE full-width residual per proj tile cover all four bands.
        qk8 = persist.tile([128, N], fp8, name="qk8")
        qkr = persist.tile([128, N], fp8, name="qkr")

        # v^T per head: chunk c occupies cols [c*VTW, c*VTW+32) (nk on
        # partitions), col c*VTW+32 is the ones column for the denominator.
        vt = [
            persist.tile([128, VTW * N_CHUNKS], bf16, name=f"vt{h}")
            for h in range(HPC)
        ]
        for h in range(HPC):
            nc.sync.dma_start(out=vt[h], in_=vones)

        # One shared PSUM pool for projection outputs and attention score
        # tiles (same 2-bank slot size).
        sc_pool = ctx.enter_context(tc.tile_pool(name="sc", bufs=3, space="PSUM"))
        ctx_pool = ctx.enter_context(tc.tile_pool(name="ctxp", bufs=2, space="PSUM"))
        ex_pool = ctx.enter_context(tc.tile_pool(name="ex", bufs=8))
        out_pool = ctx.enter_context(tc.tile_pool(name="outp", bufs=4))

        with tc.tile_pool(name="inp", bufs=1) as inp_pool:
            qin_sb = inp_pool.tile([128, N], bf16)
            kin_sb = inp_pool.tile([128, N], bf16)
            # alternate chunks across the HWDGE (sync) and SWDGE (gpsimd)
            # queues so q and k stream in concurrently
            for t in range(4):
                eng_q = nc.sync if t % 2 == 0 else nc.gpsimd
                eng_k = nc.gpsimd if t % 2 == 0 else nc.sync
                eng_q.dma_start(
                    out=qin_sb[:, ts(t, N // 4)], in_=qin[:, ts(t, N // 4)]
                )
                eng_k.dma_start(
                    out=kin_sb[:, ts(t, N // 4)], in_=kin[:, ts(t, N // 4)]
                )

            pp = sc_pool
            # ---- q/k projections + fp8 drain ----
            # psum rows: 0-31 q'_h0, 32-63 q'_h1, 64-95 k_h0, 96-127 k_h1
            for t2 in range(N // (2 * NQB)):
                qk = pp.tile([128, 2 * NQB], f32, name="qk", tag="sc")
                for j in range(2):
                    cols = ts(2 * t2 + j, NQB)
                    nc.tensor.matmul(
                        out=qk[0:64, ts(j, NQB)],
                        lhsT=wa_sb[:, 0:64],
                        rhs=qin_sb[:, cols],
                        start=True,
                        stop=True,
                    )
                    nc.tensor.matmul(
                        out=qk[64:128, ts(j, NQB)],
                        lhsT=wa_sb[:, 64:128],
                        rhs=kin_sb[:, cols],
                        start=True,
                        stop=True,
                    )
                dcols = ts(t2, 2 * NQB)
                # one convert + one residual for all four 32-row bands
                nc.scalar.copy(qk8[:, dcols], qk)
                nc.vector.tensor_tensor(
                    qkr[:, dcols], qk, qk8[:, dcols], op=ALU.subtract
                )
                # fan-out copies into the matmul operand structures
                # (SBUF->SBUF fp8; spread over Pool/ACT/DVE)
                for h in range(HPC):
                    p0 = 64 * h
                    q8s = qk8[32 * h : 32 * h + 32, dcols]
                    qrs = qkr[32 * h : 32 * h + 32, dcols]
                    k8s = qk8[64 + 32 * h : 64 + 32 * h + 32, dcols]
                    krs = qkr[64 + 32 * h : 64 + 32 * h + 32, dcols]
                    pool = nc.gpsimd.tensor_copy
                    pool(qmov[p0 : p0 + 32, dcols], q8s)
                    pool(qmov[p0 + 32 : p0 + 64, dcols], qrs)
                    nc.scalar.copy(kst[p0 : p0 + 32, 0, dcols], k8s)
                    if h == 0:
                        pool(kst[p0 + 32 : p0 + 64, 0, dcols], k8s)
                        nc.vector.tensor_copy(kst[p0 : p0 + 32, 1, dcols], krs)
                    else:
                        nc.scalar.copy(kst[p0 + 32 : p0 + 64, 0, dcols], k8s)
                        nc.vector.tensor_copy(kst[p0 : p0 + 32, 1, dcols], krs)

            # ---- v^T: out[nk, c_out] = sum_c key[c, nk] * Wv[c_out, c] ----
            # 16 chunks of [128, 64] fill one 2-bank psum tile; copy per head.
            for g in range(N_CHUNKS // 16):
                vp = pp.tile([128, 2 * NQB], f32, name="vp", tag="sc")
                for j in range(16):
                    c = g * 16 + j
                    nc.tensor.matmul(
                        out=vp[:, ts(j, 64)],
                        lhsT=kin_sb[:, ts(c, NKC)],
                        rhs=wv_sb,
                        start=True,
                        stop=True,
                    )
                vp3 = vp.rearrange("p (j w) -> p j w", j=16)
                for h in range(HPC):
                    dst = vt[h][:, g * 16 * VTW : (g + 1) * 16 * VTW]
                    dst3 = dst.rearrange("p (j w) -> p j w", j=16)
                    eng = nc.vector.tensor_copy if h % 2 == 0 else nc.scalar.copy
                    eng(dst3[:, :, 0:32], vp3[:, :, ts(h, 32)])

        # ---- attention ----
        # sc tiles span 2 PSUM banks (2 nk chunks); one exp instruction
        # drains both chunks to amortize the per-op overhead.
        #
        # Software-pipelined depth 2: the QK matmuls for score tile i are
        # emitted BEFORE the exp+PV of tile i-2, so the (in-order) PE always
        # has independent QK work while ACT/DVE drain earlier tiles — the exp
        # drain latency (~1.3us incl. semaphore hops) spans two iterations of
        # PE work. exp alternates ACT (exact, even tiles) / DVE (Schraudolph,
        # odd tiles) so each engine gets two iterations per op.
        ln2 = math.log(2.0)
        from collections import deque

        pipeline = deque()  # (sc, h, b, c2, ctx_ps) awaiting exp+PV

        def flush_one():
            sc, h, b, c2, ctx_ps = pipeline.popleft()
            ex = ex_pool.tile([128, 2 * NQB], bf16, name="ex")
            if c2 % 2 == 1:
                # fast exp2 on DVE: i16 <- sc*128 + bias; bits = bf16
                nc.vector.tensor_scalar(
                    ex.bitcast(i16), sc, EXP2_A, EXP2_B,
                    op0=ALU.mult, op1=ALU.add,
                )
            else:
                # exact exp on ACT: exp(ln2 * y) = 2^y
                nc.scalar.activation(ex, sc, AF.Exp, scale=ln2)
            for j in range(2):
                # ctxT[d, nq] += v^T[d, nk] @ probsT[nk, nq]; row 32
                # accumulates the softmax denominator (ones column).
                c = 2 * c2 + j
                nc.tensor.matmul(
                    out=ctx_ps,
                    lhsT=vt[h][:, c * VTW : (c + 1) * VTW],
                    rhs=ex[:, ts(j, NQB)],
                    start=(c == 0),
                    stop=(c == N_CHUNKS - 1),
                )
            if c2 == N_CHUNKS // 2 - 1:
                ob = out_pool.tile([33, NQB], f32, name="ob")
                nc.scalar.copy(ob, ctx_ps[0:33, :])
                nc.sync.dma_start(out=out_ctx[ts(h, 32), ts(b, NQB)], in_=ob[0:32, :])
                nc.sync.dma_start(
                    out=out_den[h : h + 1, ts(b, NQB)], in_=ob[32:33, :]
                )

        for h in range(HPC):
            p0 = 64 * h
            for b in range(N_BLOCKS):
                ctx_ps = ctx_pool.tile([VTW, NQB], f32, name="ctx_ps")
                rhs_q = (
                    qmov[p0 : p0 + 64, ts(b, NQB)]
                    .unsqueeze(1)
                    .broadcast_to([64, 2, NQB])
                )
                for c2 in range(N_CHUNKS // 2):
                    sc = sc_pool.tile([128, 2 * NQB], f32, name="sc")
                    for j in range(2):
                        c = 2 * c2 + j
                        # scoresT[nk_chunk, nq_block] via compensated fp8
                        nc.tensor.matmul(
                            out=sc[:, ts(j, NQB)],
                            lhsT=kst[p0 : p0 + 64, :, ts(c, NKC)],
                            rhs=rhs_q,
                            start=True,
                            stop=True,
                            perf_mode=DR,
                        )
                    pipeline.append((sc, h, b, c2, ctx_ps))
                    if len(pipeline) > 2:
                        flush_one()
        while pipeline:
            flush_one()

    nc.compile()
    return nc


def _shard_inputs(query, key, Wq, Wk, Wv):
    query = _f32(query).reshape(B, C, N)
    key = _f32(key).reshape(B, C, N)
    Wq, Wk, Wv = _f32(Wq), _f32(Wk), _f32(Wv)

    scale = math.log2(math.e) / math.sqrt(DH)
    in_maps = []
    for core in range(NCORES):
        b, half = core // 2, core % 2
        w_all = np.zeros((128, 128), np.float32)
        wv_t = np.zeros((128, 64), np.float32)
        for hl in range(HPC):
            ch0 = 64 * half + 32 * hl
            w_all[:, 32 * hl : 32 * hl + 32] = Wq[ch0 : ch0 + 32, :].T * scale
            w_all[:, 64 + 32 * hl : 64 + 32 * hl + 32] = Wk[ch0 : ch0 + 32, :].T
            wv_t[:, 32 * hl : 32 * hl + 32] = Wv[ch0 : ch0 + 32, :].T
        import ml_dtypes

        in_maps.append(
            {
                "qin": _bf16(query[b]),
                "kin": _bf16(key[b]),
                "w_all": _bf16(w_all),
                "wv_t": _bf16(wv_t),
                "zz": np.zeros((32, N), dtype=ml_dtypes.float8_e4m3),
                "vones": _bf16(np.ones((128, VTW * N_CHUNKS), np.float32)),
            }
        )
    return in_maps


def _run(in_maps, trace=False):
    from concourse import bass_utils

    nc = _build_program()
    return bass_utils.run_bass_kernel_spmd(
        nc, in_maps, core_ids=list(range(NCORES)), trace=trace
    )


def _assemble(results):
    out = np.empty((B, C, N), np.float32)
    for core in range(NCORES):
        b, half = core // 2, core % 2
        r = results[core]
        ctx = r["out_ctx"]  # [64, N]
        den = r["out_den"]  # [2, N]
        for hl in range(HPC):
            out[b, 64 * half + 32 * hl : 64 * half + 32 * hl + 32, :] = (
                ctx[32 * hl : 32 * hl + 32, :] / den[hl][None, :]
            )
    return out.reshape(B, C, HS, WS)


def kernel(query, key, Wq, Wk, Wv):
    in_maps = _shard_inputs(query, key, Wq, Wk, Wv)
    res = _run(in_maps)
    return _assemble(res.results)


# revision 18
# speedup vs baseline: 1.4019x; 1.0154x over previous
"""Cross-attention kernel for Trainium2 (8 NeuronCores, SPMD data-parallel).

Problem: B=4, C=128, 64x64 spatial (N=4096 tokens), 4 heads of dim 32.
  q = Wq @ query; k = Wk @ key; v = Wv @ key   (1x1 convs == channel matmuls)
  out = softmax(q^T k / sqrt(32)) @ v          (per batch*head)

Sharding: 16 (batch, head) jobs -> 2 per core. Core i handles batch i//2,
heads {2*(i%2), 2*(i%2)+1} i.e. output channels [64*(i%2), 64*(i%2)+64).

On-chip layout ("scoresT"): scores are computed transposed, [nk, nq], so that
the PV matmul needs no transposes and the context comes out directly in
channel-major [d, nq] output layout. The softmax denominator is computed by
appending a ones-column to v^T (rides the PV accumulation for free); the
final division happens on the host (softmax is scale-invariant so exp-max
subtraction is unnecessary: scores ~ N(0,1)).

QK^T runs in fp8 (e4m3) with MatmulPerfMode.DoubleRow: 0.5 cycles/row, i.e.
2x bf16 streaming rate. Accuracy is recovered by error compensation: with
q = q8 + qr and k = k8 + kr (fp8 value + fp8 residual), the two DoubleRow
accumulation slots compute
    slot0: [k8; k8]^T [q8; qr] = k8^T q8 + k8^T qr
    slot1: [kr;  0]^T [q8; * ] = kr^T q8
dropping only the O(3e-4) kr^T qr term. Measured score error: ~2.4e-3
(natural log domain) vs ~0.037 for uncompensated fp8.

PV stays bf16 (fp8 probs would add ~3.4% output error; the DoubleRow slots
can't carry a probs residual without an extra elementwise pass, and the
PSUM->SBUF drain engines are the other near-critical resource).

exp() is split between ScalarE (exact spline exp -> bf16) and VectorE
(single-op Schraudolph exp2: int16 <- y*128 + B, bits reinterpreted as bf16),
because the PSUM->SBUF drain of the 33.5M score elements per core is a
throughput floor shared by only these two engines (GPSIMD can't read PSUM).
"""

import functools
import math

import numpy as np

NCORES = 8
B, C, HS, WS = 4, 128, 64, 64
N = HS * WS  # 4096 tokens
NUM_HEADS = 4
DH = 32  # head dim
HPC = 2  # heads per core

NQB = 512  # nq block (PSUM bank = 512 f32)
NKC = 128  # nk chunk (matmul M tile)
N_BLOCKS = N // NQB  # 8
N_CHUNKS = N // NKC  # 32
VTW = 66  # v^T tile width: 32 v cols + 1 ones col + pad to >64 so that
#            round_up(M)=128 keeps every matmul in the same 128x128 PE mode

# Schraudolph exp2 in bf16: i16 = cvt(y*128 + (16256 - C)); bits = bf16 ~ 2^y
# scores are pre-scaled by log2(e)/sqrt(DH) on the Wq side so y is in log2
# domain. ACT chunks use Exp with scale=ln(2) to undo the log2 scaling.
EXP2_A = 128.0
EXP2_B = 16256.0 - 5.25
N_ACT = 8  # of the 16 score tiles per (h, b): 8 exact-exp (ACT), 8 DVE


def _f32(x):
    return np.ascontiguousarray(np.asarray(x, dtype=np.float32))


def _bf16(x):
    import ml_dtypes

    return np.ascontiguousarray(np.asarray(x, dtype=np.float32).astype(ml_dtypes.bfloat16))


@functools.lru_cache(maxsize=1)
def _build_program():
    from contextlib import ExitStack

    import concourse.tile as tile
    from concourse import bacc, mybir
    from concourse.bass import ts

    f32 = mybir.dt.float32
    bf16 = mybir.dt.bfloat16
    fp8 = mybir.dt.float8e4
    i16 = mybir.dt.int16
    AF = mybir.ActivationFunctionType
    ALU = mybir.AluOpType
    DR = mybir.MatmulPerfMode.DoubleRow

    nc = bacc.Bacc(
        "TRN2",
        target_bir_lowering=False,
        debug=False,
        enable_asserts=False,
        num_devices=NCORES,
    )

    qin = nc.dram_tensor("qin", [128, N], bf16, kind="ExternalInput").ap()
    kin = nc.dram_tensor("kin", [128, N], bf16, kind="ExternalInput").ap()
    # w_all: projection stationary [c_in=128, c_out=128] (host-prepared):
    # cols 0-31 q_h0 (scaled by log2e/sqrt(DH)), 32-63 q_h1, 64-95 k_h0,
    # 96-127 k_h1.
    w_all = nc.dram_tensor("w_all", [128, 128], bf16, kind="ExternalInput").ap()
    wv_t = nc.dram_tensor("wv_t", [128, 64], bf16, kind="ExternalInput").ap()

    # constant fills, DMA'd instead of burning engine memset time at startup
    zz = nc.dram_tensor("zz", [32, N], fp8, kind="ExternalInput").ap()
    vones = nc.dram_tensor(
        "vones", [128, VTW * N_CHUNKS], bf16, kind="ExternalInput"
    ).ap()

    out_ctx = nc.dram_tensor("out_ctx", [64, N], f32, kind="ExternalOutput").ap()
    out_den = nc.dram_tensor("out_den", [2, N], f32, kind="ExternalOutput").ap()

    with tile.TileContext(nc) as tc, ExitStack() as ctx:
        persist = ctx.enter_context(tc.tile_pool(name="persist", bufs=1))

        # ---- load inputs ----
        wa_sb = persist.tile([128, 128], bf16)
        wv_sb = persist.tile([128, 64], bf16)
        nc.sync.dma_start(out=wa_sb, in_=w_all)
        nc.sync.dma_start(out=wv_sb, in_=wv_t)

        # ---- fp8 q/k structures ----
        # qmov (moving): rows 64h..64h+32 = q8_h, rows +32..64 = qr_h; the
        # DoubleRow rhs broadcasts the slot dim with stride 0 (slot1's qr
        # rows meet zero rows of the stationary, so they're don't-cares).
        # kst (stationary): rows 64h..+32 slot0 = k8_h, +32..64 slot0 = k8_h
        # again; slot1 = [kr_h; 0] (zero bands DMA'd from zz).
        qmov = persist.tile([128, N], fp8, name="qmov")
        kst = persist.tile([128, 2, N], fp8, name="kst")
        # fp8 staging: qk8 = fp8(proj), qkr = fp8(proj - qk8), both in proj
        # row layout [q_h0, q_h1, k_h0, k_h1] so that ONE full-width convert
        # and ONE full-width residual per proj tile cover all four bands.
        qk8 = persist.tile([128, N], fp8, name="qk8")
        qkr = persist.tile([128, N], fp8, name="qkr")

        # v^T per head: chunk c occupies cols [c*VTW, c*VTW+32) (nk on
        # partitions), col c*VTW+32 is the ones column for the denominator.
        vt = [
            persist.tile([128, VTW * N_CHUNKS], bf16, name=f"vt{h}")
            for h in range(HPC)
        ]
        # ones fill on the (idle-at-startup) compute engines; zero bands of
        # kst slot1 via DMA (big enough that engine memset would be slow)
        nc.vector.memset(vt[0], 1.0)
        nc.gpsimd.memset(vt[1], 1.0)

        # One shared PSUM pool for projection outputs and attention score
        # tiles (same 2-bank slot size).
        sc_pool = ctx.enter_context(tc.tile_pool(name="sc", bufs=3, space="PSUM"))
        ctx_pool = ctx.enter_context(tc.tile_pool(name="ctxp", bufs=2, space="PSUM"))
        ex_pool = ctx.enter_context(tc.tile_pool(name="ex", bufs=8))
        out_pool = ctx.enter_context(tc.tile_pool(name="outp", bufs=4))

        with tc.tile_pool(name="inp", bufs=1) as inp_pool:
            qin_sb = inp_pool.tile([128, N], bf16)
            kin_sb = inp_pool.tile([128, N], bf16)
            # kin lands first (v-projection and the k-side of q/k projection
            # both need it); chunks split across HWDGE (sync) and SWDGE
            # (gpsimd) queues.
            for t in range(2):
                nc.sync.dma_start(
                    out=kin_sb[:, ts(t, N // 4)], in_=kin[:, ts(t, N // 4)]
                )
                nc.gpsimd.dma_start(
                    out=kin_sb[:, ts(2 + t, N // 4)], in_=kin[:, ts(2 + t, N // 4)]
                )
            for t in range(2):
                nc.sync.dma_start(
                    out=qin_sb[:, ts(t, N // 4)], in_=qin[:, ts(t, N // 4)]
                )
                nc.gpsimd.dma_start(
                    out=qin_sb[:, ts(2 + t, N // 4)], in_=qin[:, ts(2 + t, N // 4)]
                )
            nc.sync.dma_start(out=kst[32:64, 1, :], in_=zz)
            nc.sync.dma_start(out=kst[96:128, 1, :], in_=zz)

            pp = sc_pool
            # ---- v^T first: needed by the first PV matmuls; only reads kin.
            # out[nk, c_out] = sum_c key[c, nk] * Wv[c_out, c]; 16 chunks of
            # [128, 64] fill one 2-bank psum tile; copy per head. ----
            for g in range(N_CHUNKS // 16):
                vp = pp.tile([128, 2 * NQB], f32, name="vp", tag="sc")
                for j in range(16):
                    c = g * 16 + j
                    nc.tensor.matmul(
                        out=vp[:, ts(j, 64)],
                        lhsT=kin_sb[:, ts(c, NKC)],
                        rhs=wv_sb,
                        start=True,
                        stop=True,
                    )
                vp3 = vp.rearrange("p (j w) -> p j w", j=16)
                for h in range(HPC):
                    dst = vt[h][:, g * 16 * VTW : (g + 1) * 16 * VTW]
                    dst3 = dst.rearrange("p (j w) -> p j w", j=16)
                    eng = nc.vector.tensor_copy if h % 2 == 0 else nc.scalar.copy
                    eng(dst3[:, :, 0:32], vp3[:, :, ts(h, 32)])

            # ---- q/k projections + fp8 drain ----
            # psum rows: 0-31 q'_h0, 32-63 q'_h1, 64-95 k_h0, 96-127 k_h1
            for t2 in range(N // (2 * NQB)):
                qk = pp.tile([128, 2 * NQB], f32, name="qk", tag="sc")
                for j in range(2):
                    cols = ts(2 * t2 + j, NQB)
                    nc.tensor.matmul(
                        out=qk[0:64, ts(j, NQB)],
                        lhsT=wa_sb[:, 0:64],
                        rhs=qin_sb[:, cols],
                        start=True,
                        stop=True,
                    )
                    nc.tensor.matmul(
                        out=qk[64:128, ts(j, NQB)],
                        lhsT=wa_sb[:, 64:128],
                        rhs=kin_sb[:, cols],
                        start=True,
                        stop=True,
                    )
                dcols = ts(t2, 2 * NQB)
                # one convert + one residual for all four 32-row bands
                nc.scalar.copy(qk8[:, dcols], qk)
                nc.vector.tensor_tensor(
                    qkr[:, dcols], qk, qk8[:, dcols], op=ALU.subtract
                )
                # fan-out copies into the matmul operand structures
                # (SBUF->SBUF fp8; spread over Pool/ACT/DVE)
                for h in range(HPC):
                    p0 = 64 * h
                    q8s = qk8[32 * h : 32 * h + 32, dcols]
                    qrs = qkr[32 * h : 32 * h + 32, dcols]
                    k8s = qk8[64 + 32 * h : 64 + 32 * h + 32, dcols]
                    krs = qkr[64 + 32 * h : 64 + 32 * h + 32, dcols]
                    pool = nc.gpsimd.tensor_copy
                    pool(qmov[p0 : p0 + 32, dcols], q8s)
                    pool(qmov[p0 + 32 : p0 + 64, dcols], qrs)
                    nc.scalar.copy(kst[p0 : p0 + 32, 0, dcols], k8s)
                    if h == 0:
                        pool(kst[p0 + 32 : p0 + 64, 0, dcols], k8s)
                        nc.vector.tensor_copy(kst[p0 : p0 + 32, 1, dcols], krs)
                    else:
                        nc.scalar.copy(kst[p0 + 32 : p0 + 64, 0, dcols], k8s)
                        nc.vector.tensor_copy(kst[p0 : p0 + 32, 1, dcols], krs)

        # ---- attention ----
        # sc tiles span 2 PSUM banks (2 nk chunks); one exp instruction
        # drains both chunks to amortize the per-op overhead.
        #
        # Software-pipelined depth 2: the QK matmuls for score tile i are
        # emitted BEFORE the exp+PV of tile i-2, so the (in-order) PE always
        # has independent QK work while ACT/DVE drain earlier tiles — the exp
        # drain latency (~1.3us incl. semaphore hops) spans two iterations of
        # PE work. exp alternates ACT (exact, even tiles) / DVE (Schraudolph,
        # odd tiles) so each engine gets two iterations per op.
        ln2 = math.log(2.0)
        from collections import deque

        pipeline = deque()  # (sc, h, b, c2, ctx_ps) awaiting exp+PV

        def flush_one():
            sc, h, b, c2, ctx_ps = pipeline.popleft()
            ex = ex_pool.tile([128, 2 * NQB], bf16, name="ex")
            if c2 % 2 == 1:
                # fast exp2 on DVE: i16 <- sc*128 + bias; bits = bf16
                nc.vector.tensor_scalar(
                    ex.bitcast(i16), sc, EXP2_A, EXP2_B,
                    op0=ALU.mult, op1=ALU.add,
                )
            else:
                # exact exp on ACT: exp(ln2 * y) = 2^y
                nc.scalar.activation(ex, sc, AF.Exp, scale=ln2)
            for j in range(2):
                # ctxT[d, nq] += v^T[d, nk] @ probsT[nk, nq]; row 32
                # accumulates the softmax denominator (ones column).
                c = 2 * c2 + j
                nc.tensor.matmul(
                    out=ctx_ps,
                    lhsT=vt[h][:, c * VTW : (c + 1) * VTW],
                    rhs=ex[:, ts(j, NQB)],
                    start=(c == 0),
                    stop=(c == N_CHUNKS - 1),
                )
            if c2 == N_CHUNKS // 2 - 1:
                ob = out_pool.tile([33, NQB], f32, name="ob")
                nc.scalar.copy(ob, ctx_ps[0:33, :])
                nc.sync.dma_start(out=out_ctx[ts(h, 32), ts(b, NQB)], in_=ob[0:32, :])
                nc.sync.dma_start(
                    out=out_den[h : h + 1, ts(b, NQB)], in_=ob[32:33, :]
                )

        for h in range(HPC):
            p0 = 64 * h
            for b in range(N_BLOCKS):
                ctx_ps = ctx_pool.tile([VTW, NQB], f32, name="ctx_ps")
                rhs_q = (
                    qmov[p0 : p0 + 64, ts(b, NQB)]
                    .unsqueeze(1)
                    .broadcast_to([64, 2, NQB])
                )
                for c2 in range(N_CHUNKS // 2):
                    sc = sc_pool.tile([128, 2 * NQB], f32, name="sc")
                    for j in range(2):
                        c = 2 * c2 + j
                        # scoresT[nk_chunk, nq_block] via compensated fp8
                        nc.tensor.matmul(
                            out=sc[:, ts(j, NQB)],
                            lhsT=kst[p0 : p0 + 64, :, ts(c, NKC)],
                            rhs=rhs_q,
                            start=True,
                            stop=True,
                            perf_mode=DR,
                        )
                    pipeline.append((sc, h, b, c2, ctx_ps))
                    if len(pipeline) > 2:
                        flush_one()
        while pipeline:
            flush_one()

    nc.compile()
    return nc


def _shard_inputs(query, key, Wq, Wk, Wv):
    query = _f32(query).reshape(B, C, N)
    key = _f32(key).reshape(B, C, N)
    Wq, Wk, Wv = _f32(Wq), _f32(Wk), _f32(Wv)

    scale = math.log2(math.e) / math.sqrt(DH)
    in_maps = []
    for core in range(NCORES):
        b, half = core // 2, core % 2
        w_all = np.zeros((128, 128), np.float32)
        wv_t = np.zeros((128, 64), np.float32)
        for hl in range(HPC):
            ch0 = 64 * half + 32 * hl
            w_all[:, 32 * hl : 32 * hl + 32] = Wq[ch0 : ch0 + 32, :].T * scale
            w_all[:, 64 + 32 * hl : 64 + 32 * hl + 32] = Wk[ch0 : ch0 + 32, :].T
            wv_t[:, 32 * hl : 32 * hl + 32] = Wv[ch0 : ch0 + 32, :].T
        import ml_dtypes

        in_maps.append(
            {
                "qin": _bf16(query[b]),
                "kin": _bf16(key[b]),
                "w_all": _bf16(w_all),
                "wv_t": _bf16(wv_t),
                "zz": np.zeros((32, N), dtype=ml_dtypes.float8_e4m3),
                "vones": _bf16(np.ones((128, VTW * N_CHUNKS), np.float32)),
            }
        )
    return in_maps


def _run(in_maps, trace=False):
    from concourse import bass_utils

    nc = _build_program()
    return bass_utils.run_bass_kernel_spmd(
        nc, in_maps, core_ids=list(range(NCORES)), trace=trace
    )


def _assemble(results):
    out = np.empty((B, C, N), np.float32)
    for core in range(NCORES):
        b, half = core // 2, core % 2
        r = results[core]
        ctx = r["out_ctx"]  # [64, N]
        den = r["out_den"]  # [2, N]
        for hl in range(HPC):
            out[b, 64 * half + 32 * hl : 64 * half + 32 * hl + 32, :] = (
                ctx[32 * hl : 32 * hl + 32, :] / den[hl][None, :]
            )
    return out.reshape(B, C, HS, WS)


def kernel(query, key, Wq, Wk, Wv):
    in_maps = _shard_inputs(query, key, Wq, Wk, Wv)
    res = _run(in_maps)
    return _assemble(res.results)
